# revision 17
# baseline (speedup 1.0000x reference)
"""GroupedPNMLP forward on 8 Trainium2 NeuronCores (pure data parallel).

Per-node 2-layer MLP (32->32->1), 24 nodes in 6 groups of 4, with a
group-validity mask and node permutation.  Full inputs in, full output out;
samples are sharded N/8 per core, tiny weights replicated.

v4 pipeline (chunks of C=2048 samples, block-cyclic sample->partition):
  DMA h in two half-chunks (24KB contiguous per partition)
  -> ReLU + downcast to bf16 on DVE (2x_2p mode), written in a
     [n, t, c, s2] swizzle so neighbouring samples (s2) pair up
  -> DVE 32x32 block-transpose on the uint32 *pair* view: half the
     elements of an elementwise transpose; channels land on partitions
  -> mm1: ONE 128x128 block-diagonal bf16 matmul per node (4 copies of
     W1n on the diagonal -> all 4 sample sub-blocks in one shot); free
     dim 512 = one PSUM bank per node
  -> hidden drain: ReLU (+b1), PSUM->SBUF bf16 on ScalarE, one ACTIVATE
     per 2-bank pair
  -> mm2: 128x128 block-diagonal W2 (column slot = node id), 24 bf16
     matmuls accumulating into one PSUM bank; the rhs access pattern
     re-enumerates samples (hh,t,s2,pi) so pi is innermost again
  -> +b2 on ScalarE, DVE transpose back, group-valid mask multiply on
     GpSimd in natural layout, DMA out
  Mask path (small) runs on GpSimd + DVE off the critical path.
"""

import numpy as np
import ml_dtypes

import concourse.bass as bass
from concourse import bacc
import concourse.tile as tile
from concourse import mybir
from concourse.bass_utils import run_bass_kernel_spmd

F32 = mybir.dt.float32
BF16 = mybir.dt.bfloat16
I32 = mybir.dt.int32
U32 = mybir.dt.uint32

GROUPING = np.array(
    [[0, 3, 6, 9], [1, 4, 7, 10], [2, 5, 8, 11],
     [12, 13, 14, 15], [16, 18, 20, 22], [17, 19, 21, 23]], dtype=np.int32)

N_CORES = 8
S_TOT = 131072
S = S_TOT // N_CORES      # 16384 samples per core
NODES = 24
CH = 32                   # in channels = hidden dim
C = 2048                  # samples per chunk
NSUB = C // 128           # 16 samples per partition per chunk
HSUB = NSUB // 2          # 8 per half-chunk (t in 0..3, s2 in 0..1)
NCHUNK = S // C           # 8
NW = 6                    # waves of 4 nodes

# input-relu t-slices (of 4) on ScalarE; rest on DVE
ACT_T = 0


def _build_program():
    nc = bacc.Bacc(None, target_bir_lowering=False)

    h = nc.dram_tensor("h", [S, NODES * CH], F32, kind="ExternalInput")
    valid = nc.dram_tensor("valid", [S, NODES], I32, kind="ExternalInput")
    w1d = nc.dram_tensor("w1d", [128, NODES, 128], BF16, kind="ExternalInput")
    w2d = nc.dram_tensor("w2d", [128, NODES, 128], BF16, kind="ExternalInput")
    b1c = nc.dram_tensor("b1c", [128, NODES], F32, kind="ExternalInput")
    b2c = nc.dram_tensor("b2c", [128, 1], F32, kind="ExternalInput")
    out = nc.dram_tensor("out", [S, NODES], F32, kind="ExternalOutput")

    with tile.TileContext(nc) as tc:
        with (
            tc.tile_pool(name="singles", bufs=1) as singles,
            tc.tile_pool(name="xp", bufs=2) as xp,
            tc.tile_pool(name="xrp", bufs=2) as xrp,
            tc.tile_pool(name="xtp", bufs=2) as xtp,
            tc.tile_pool(name="hp", bufs=3) as hp,
            tc.tile_pool(name="vp", bufs=4) as vp,
            tc.tile_pool(name="op", bufs=4) as op,
            tc.tile_pool(name="php", bufs=3, space="PSUM") as php,
            tc.tile_pool(name="p2p", bufs=2, space="PSUM") as p2p,
        ):
            w1sb = singles.tile([128, NODES, 128], BF16)
            nc.scalar.dma_start(out=w1sb, in_=w1d[:, :, :])
            w2sb = singles.tile([128, NODES, 128], BF16)
            nc.scalar.dma_start(out=w2sb, in_=w2d[:, :, :])
            b1sb = singles.tile([128, NODES], F32)
            nc.scalar.dma_start(out=b1sb, in_=b1c[:, :])
            b2sb = singles.tile([128, 1], F32)
            nc.scalar.dma_start(out=b2sb, in_=b2c[:, :])

            def input_phase(cc):
                """DMA h, relu+cast (DVE), pair-transpose, mask prep.
                Returns (xt, mk)."""
                c0 = cc * C
                # valid -> group mask first (small, independent; keeps the
                # DVE/GpSimd queues from blocking on it mid-chunk)
                vi = vp.tile([128, NSUB, NODES], I32)
                for hh in range(2):
                    lo = c0 + hh * (C // 2)
                    nc.gpsimd.dma_start(
                        out=vi[:, hh * HSUB:(hh + 1) * HSUB],
                        in_=valid[lo:lo + C // 2, :].rearrange(
                            "(p s) n -> p s n", p=128),
                    )
                vf = vp.tile([128, NSUB, NODES], F32)
                nc.vector.tensor_copy(vf, vi)
                gv = vp.tile([128, NSUB, 8], F32)
                nc.vector.tensor_reduce(
                    gv[:, :, 0:3],
                    vf[:, :, 0:12].rearrange("p s (k g) -> p s g k", g=3),
                    axis=mybir.AxisListType.X, op=mybir.AluOpType.add)
                nc.vector.tensor_reduce(
                    gv[:, :, 3:4], vf[:, :, 12:16],
                    axis=mybir.AxisListType.X, op=mybir.AluOpType.add)
                nc.vector.tensor_reduce(
                    gv[:, :, 4:6],
                    vf[:, :, 16:24].rearrange("p s (k g) -> p s g k", g=2),
                    axis=mybir.AxisListType.X, op=mybir.AluOpType.add)
                nc.vector.tensor_scalar(
                    gv[:, :, 0:6], gv[:, :, 0:6], 0.5, None,
                    op0=mybir.AluOpType.is_gt)
                mk = vp.tile([128, NSUB, NODES], F32)
                nc.gpsimd.tensor_copy(
                    mk[:, :, 0:12].rearrange("p s (k g) -> p s g k", g=3),
                    gv[:, :, 0:3].broadcast_to([128, NSUB, 3, 4]))
                nc.gpsimd.tensor_copy(
                    mk[:, :, 12:16],
                    gv[:, :, 3:4].broadcast_to([128, NSUB, 4]))
                nc.gpsimd.tensor_copy(
                    mk[:, :, 16:24].rearrange("p s (k g) -> p s g k", g=2),
                    gv[:, :, 4:6].broadcast_to([128, NSUB, 2, 4]))

                # xt[32b+c, hh, n, t, 2*pi+s2] = relu(h[sample, n, c]) where
                # sample = c0 + hh*1024 + (32b+pi)*8 + 2*t + s2
                xt = xtp.tile([128, 2, NODES, 4, 64], BF16)
                for hh in range(2):
                    xh = xp.tile([128, HSUB, NODES, CH], F32)
                    lo = c0 + hh * (C // 2)
                    nc.sync.dma_start(
                        out=xh.rearrange("p s n c -> p (s n c)"),
                        in_=h[lo:lo + C // 2, :].rearrange(
                            "(p s) f -> p (s f)", p=128),
                    )
                    # xr[p, n, t, (c, s2)] = relu(xh[p, 2t+s2, n, c]) bf16
                    # (3D-AP limit: one instruction per s2 phase)
                    xr = xrp.tile([128, NODES, 4, 64], BF16)
                    xr_v = xr.rearrange("p n t (c s2) -> p s2 t n c", s2=2)
                    xh_v = xh.rearrange("p (t s2) n c -> p s2 t n c", s2=2)
                    for s2 in range(2):
                        if ACT_T > 0:
                            nc.scalar.activation(
                                xr_v[:, s2, 0:ACT_T], xh_v[:, s2, 0:ACT_T],
                                mybir.ActivationFunctionType.Relu)
                        if ACT_T < 4:
                            nc.vector.tensor_scalar_max(
                                xr_v[:, s2, ACT_T:4], xh_v[:, s2, ACT_T:4],
                                0.0)
                    # u32 pair transpose: channels onto partitions
                    nc.vector.transpose(
                        xt[:, hh].bitcast(U32), xr.bitcast(U32))
                return xt, mk

            def mlp_phase(xt):
                """Block-diag 128x128 matmuls; returns p2 (PSUM)."""
                p2 = p2p.tile([128, NSUB, CH], F32)   # ((ht, s2), pi)
                p2f = p2.rearrange("p a b -> p (a b)")

                def emit_mm2(hidt, n0):
                    for q in range(2):
                        n = n0 + q
                        nc.tensor.matmul(
                            p2f,
                            lhsT=w2sb[:, n, :],
                            rhs=hidt[:, q].rearrange(
                                "p (ht pi s2) -> p ht s2 pi",
                                ht=8, s2=2),
                            start=(n == 0), stop=(n == NODES - 1),
                            skip_group_check=True)

                # software-pipelined: mm2 of half-wave k emits after the
                # mm1s of half-wave k+2, so drains never block the PE queue
                pend = []
                for k in range(2 * NW):
                    n0 = 2 * k
                    ph = php.tile([128, 2, 512], F32)
                    hidt = hp.tile([128, 2, 512], BF16)
                    for q in range(2):
                        nc.tensor.matmul(
                            ph[:, q, :],
                            lhsT=w1sb[:, n0 + q, :],
                            rhs=xt[:, :, n0 + q, :, :],
                            start=True, stop=True)
                    nc.scalar.activation(
                        hidt.rearrange("p a b -> p (a b)"),
                        ph.rearrange("p a b -> p (a b)"),
                        mybir.ActivationFunctionType.Relu,
                        bias=b1sb[:, n0:n0 + 1])
                    pend.append((hidt, n0))
                    if len(pend) > 2:
                        emit_mm2(*pend.pop(0))
                for args in pend:
                    emit_mm2(*args)
                return p2

            def output_phase(cc, p2, mk):
                """+b2, transpose back, mask, store chunk cc."""
                c0 = cc * C
                # p2 block f = (ht, s2) enumerates sub = hh*8 + 2t + s2
                # sequentially, so outT free is natural (sub, node) order.
                m2 = op.tile([128, NSUB, CH], F32)
                nc.scalar.activation(
                    m2.rearrange("p a b -> p (a b)"),
                    p2.rearrange("p a b -> p (a b)"),
                    mybir.ActivationFunctionType.Identity,
                    bias=b2sb[:, 0:1])
                outT = op.tile([128, NSUB, CH], F32)
                nc.vector.transpose(outT, m2)
                outF = op.tile([128, NSUB, NODES], F32)
                nc.gpsimd.tensor_tensor(
                    outF, outT[:, :, 0:NODES], mk, op=mybir.AluOpType.mult)
                for hh in range(2):
                    lo = c0 + hh * (C // 2)
                    nc.gpsimd.dma_start(
                        out=out[lo:lo + C // 2, :].rearrange(
                            "(p s) n -> p s n", p=128),
                        in_=outF[:, hh * HSUB:(hh + 1) * HSUB],
                    )

            # chunk-level software pipeline: the output phase of chunk
            # cc-1 is emitted between input(cc) and mlp(cc) so DVE's
            # in-order queue never blocks input work behind an output
            # transpose that depends on the previous chunk's full MLP.
            prev = None
            for cc in range(NCHUNK):
                xt, mk = input_phase(cc)
                if prev is not None:
                    output_phase(cc - 1, prev[0], prev[1])
                p2 = mlp_phase(xt)
                prev = (p2, mk)
            output_phase(NCHUNK - 1, prev[0], prev[1])

    nc.compile()
    return nc


_PROGRAM = None


def _get_program():
    global _PROGRAM
    if _PROGRAM is None:
        _PROGRAM = _build_program()
    return _PROGRAM


def _prep_weights(W1, b1, W2, b2):
    flat = GROUPING.reshape(-1)
    g_of = np.zeros(NODES, np.int64)
    k_of = np.zeros(NODES, np.int64)
    for q, nid in enumerate(flat):
        g_of[nid] = q // 4
        k_of[nid] = q % 4
    W1n = np.ascontiguousarray(W1[g_of, k_of]).astype(np.float32)  # [24,32,32]
    W2n = np.ascontiguousarray(W2[g_of, k_of]).astype(np.float32)  # [24,32,1]
    b1n = np.ascontiguousarray(b1[g_of, k_of]).astype(np.float32)  # [24,32]
    b2n = np.ascontiguousarray(b2[g_of, k_of]).astype(np.float32)  # [24,1]

    w1dv = np.zeros((128, NODES, 128), np.float32)
    w2dv = np.zeros((128, NODES, 128), np.float32)
    b1cv = np.zeros((128, NODES), np.float32)
    b2cv = np.zeros((128, 1), np.float32)
    for n in range(NODES):
        for i in range(4):
            sl = slice(32 * i, 32 * i + 32)
            w1dv[sl, n, sl] = W1n[n]                       # [c, h] block
            w2dv[sl, n, 32 * i + n] = W2n[n][:, 0]         # column slot n
            b1cv[sl, n] = b1n[n]
            b2cv[32 * i + n, 0] = b2n[n, 0]
    return (w1dv.astype(ml_dtypes.bfloat16), w2dv.astype(ml_dtypes.bfloat16),
            b1cv, b2cv)


def _make_in_maps(inputs):
    w1dv, w2dv, b1cv, b2cv = _prep_weights(
        inputs["W1"], inputs["b1"], inputs["W2"], inputs["b2"])
    h2 = np.ascontiguousarray(inputs["h"], dtype=np.float32).reshape(
        S_TOT, NODES * CH)
    v2 = np.ascontiguousarray(inputs["valid"], dtype=np.int32).reshape(
        S_TOT, NODES)
    in_maps = []
    for c in range(N_CORES):
        sl = slice(c * S, (c + 1) * S)
        in_maps.append({
            "h": h2[sl],
            "valid": v2[sl],
            "w1d": w1dv,
            "w2d": w2dv,
            "b1c": b1cv,
            "b2c": b2cv,
        })
    return in_maps


def kernel(h, valid, W1, b1, W2, b2):
    nc = _get_program()
    in_maps = _make_in_maps(
        {"h": h, "valid": valid, "W1": W1, "b1": b1, "W2": W2, "b2": b2})
    res = run_bass_kernel_spmd(nc, in_maps, core_ids=list(range(N_CORES)))
    outs = [res.results[c]["out"] for c in range(N_CORES)]
    full = np.concatenate(outs, axis=0).astype(np.float32)
    return full.reshape(S_TOT, NODES, 1)


# revision 19
# speedup vs baseline: 1.0343x; 1.0343x over previous
"""GroupedPNMLP forward on 8 Trainium2 NeuronCores (pure data parallel).

Per-node 2-layer MLP (32->32->1), 24 nodes in 6 groups of 4, with a
group-validity mask and node permutation.  Full inputs in, full output out;
samples are sharded N/8 per core, tiny weights replicated.

v4 pipeline (chunks of C=2048 samples, block-cyclic sample->partition):
  DMA h in two half-chunks (24KB contiguous per partition)
  -> ReLU + downcast to bf16 on DVE (2x_2p mode), written in a
     [n, t, c, s2] swizzle so neighbouring samples (s2) pair up
  -> DVE 32x32 block-transpose on the uint32 *pair* view: half the
     elements of an elementwise transpose; channels land on partitions
  -> mm1: ONE 128x128 block-diagonal bf16 matmul per node (4 copies of
     W1n on the diagonal -> all 4 sample sub-blocks in one shot); free
     dim 512 = one PSUM bank per node
  -> hidden drain: ReLU (+b1), PSUM->SBUF bf16 on ScalarE, one ACTIVATE
     per 2-bank pair
  -> mm2: 128x128 block-diagonal W2 (column slot = node id), 24 bf16
     matmuls accumulating into one PSUM bank; the rhs access pattern
     re-enumerates samples (hh,t,s2,pi) so pi is innermost again
  -> +b2 on ScalarE, DVE transpose back, group-valid mask multiply on
     GpSimd in natural layout, DMA out
  Mask path (small) runs on GpSimd + DVE off the critical path.
"""

import numpy as np
import ml_dtypes

import concourse.bass as bass
from concourse import bacc
import concourse.tile as tile
from concourse import mybir
from concourse.bass_utils import run_bass_kernel_spmd

F32 = mybir.dt.float32
BF16 = mybir.dt.bfloat16
I32 = mybir.dt.int32
U32 = mybir.dt.uint32

GROUPING = np.array(
    [[0, 3, 6, 9], [1, 4, 7, 10], [2, 5, 8, 11],
     [12, 13, 14, 15], [16, 18, 20, 22], [17, 19, 21, 23]], dtype=np.int32)

N_CORES = 8
S_TOT = 131072
S = S_TOT // N_CORES      # 16384 samples per core
NODES = 24
CH = 32                   # in channels = hidden dim
C = 2048                  # samples per chunk
NSUB = C // 128           # 16 samples per partition per chunk
HSUB = NSUB // 2          # 8 per half-chunk (t in 0..3, s2 in 0..1)
NCHUNK = S // C           # 8
NW = 6                    # waves of 4 nodes

# input-relu t-slices (of 4) on ScalarE; rest on DVE
ACT_T = 0


def _build_program():
    nc = bacc.Bacc(None, target_bir_lowering=False)

    h = nc.dram_tensor("h", [S, NODES * CH], F32, kind="ExternalInput")
    valid = nc.dram_tensor("valid", [S, NODES], I32, kind="ExternalInput")
    w1d = nc.dram_tensor("w1d", [128, NODES, 128], BF16, kind="ExternalInput")
    w2d = nc.dram_tensor("w2d", [128, NODES, 128], BF16, kind="ExternalInput")
    b1c = nc.dram_tensor("b1c", [128, NODES], F32, kind="ExternalInput")
    b2c = nc.dram_tensor("b2c", [128, 1], F32, kind="ExternalInput")
    out = nc.dram_tensor("out", [S, NODES], F32, kind="ExternalOutput")

    with tile.TileContext(nc) as tc:
        with (
            tc.tile_pool(name="singles", bufs=1) as singles,
            tc.tile_pool(name="xp", bufs=2) as xp,
            tc.tile_pool(name="xrp", bufs=2) as xrp,
            tc.tile_pool(name="xtp", bufs=2) as xtp,
            tc.tile_pool(name="hp", bufs=3) as hp,
            tc.tile_pool(name="vp", bufs=2) as vp,
            tc.tile_pool(name="op", bufs=2) as op,
            tc.tile_pool(name="php", bufs=3, space="PSUM") as php,
            tc.tile_pool(name="p2p", bufs=2, space="PSUM") as p2p,
        ):
            w1sb = singles.tile([128, NODES, 128], BF16)
            nc.sync.dma_start(out=w1sb, in_=w1d[:, :, :])
            w2sb = singles.tile([128, NODES, 128], BF16)
            nc.sync.dma_start(out=w2sb, in_=w2d[:, :, :])
            b1sb = singles.tile([128, NODES], F32)
            nc.sync.dma_start(out=b1sb, in_=b1c[:, :])
            b2sb = singles.tile([128, 1], F32)
            nc.sync.dma_start(out=b2sb, in_=b2c[:, :])

            def input_phase(cc):
                """DMA h, relu+cast (DVE), pair-transpose, mask prep.
                Returns (xt, mk)."""
                c0 = cc * C
                # xt[32b+c, hh, n, t, 2*pi+s2] = relu(h[sample, n, c]) where
                # sample = c0 + hh*1024 + (32b+pi)*8 + 2*t + s2
                xt = xtp.tile([128, 2, NODES, 4, 64], BF16)
                for hh in range(2):
                    xh = xp.tile([128, HSUB, NODES, CH], F32)
                    lo = c0 + hh * (C // 2)
                    nc.sync.dma_start(
                        out=xh.rearrange("p s n c -> p (s n c)"),
                        in_=h[lo:lo + C // 2, :].rearrange(
                            "(p s) f -> p (s f)", p=128),
                    )
                    # xr[p, n, t, (c, s2)] = relu(xh[p, 2t+s2, n, c]) bf16
                    # (3D-AP limit: one instruction per s2 phase)
                    xr = xrp.tile([128, NODES, 4, 64], BF16)
                    xr_v = xr.rearrange("p n t (c s2) -> p s2 t n c", s2=2)
                    xh_v = xh.rearrange("p (t s2) n c -> p s2 t n c", s2=2)
                    for s2 in range(2):
                        if ACT_T > 0:
                            nc.scalar.activation(
                                xr_v[:, s2, 0:ACT_T], xh_v[:, s2, 0:ACT_T],
                                mybir.ActivationFunctionType.Relu)
                        if ACT_T < 4:
                            nc.vector.tensor_scalar_max(
                                xr_v[:, s2, ACT_T:4], xh_v[:, s2, ACT_T:4],
                                0.0)
                    # u32 pair transpose: channels onto partitions
                    nc.vector.transpose(
                        xt[:, hh].bitcast(U32), xr.bitcast(U32))

                # valid -> group mask (natural layout, off crit path)
                vi = vp.tile([128, NSUB, NODES], I32)
                for hh in range(2):
                    lo = c0 + hh * (C // 2)
                    nc.gpsimd.dma_start(
                        out=vi[:, hh * HSUB:(hh + 1) * HSUB],
                        in_=valid[lo:lo + C // 2, :].rearrange(
                            "(p s) n -> p s n", p=128),
                    )
                vf = vp.tile([128, NSUB, NODES], F32)
                nc.vector.tensor_copy(vf, vi)
                gv = vp.tile([128, NSUB, 8], F32)
                nc.vector.tensor_reduce(
                    gv[:, :, 0:3],
                    vf[:, :, 0:12].rearrange("p s (k g) -> p s g k", g=3),
                    axis=mybir.AxisListType.X, op=mybir.AluOpType.add)
                nc.vector.tensor_reduce(
                    gv[:, :, 3:4], vf[:, :, 12:16],
                    axis=mybir.AxisListType.X, op=mybir.AluOpType.add)
                nc.vector.tensor_reduce(
                    gv[:, :, 4:6],
                    vf[:, :, 16:24].rearrange("p s (k g) -> p s g k", g=2),
                    axis=mybir.AxisListType.X, op=mybir.AluOpType.add)
                nc.vector.tensor_scalar(
                    gv[:, :, 0:6], gv[:, :, 0:6], 0.5, None,
                    op0=mybir.AluOpType.is_gt)
                mk = vp.tile([128, NSUB, NODES], F32)
                nc.gpsimd.tensor_copy(
                    mk[:, :, 0:12].rearrange("p s (k g) -> p s g k", g=3),
                    gv[:, :, 0:3].broadcast_to([128, NSUB, 3, 4]))
                nc.gpsimd.tensor_copy(
                    mk[:, :, 12:16],
                    gv[:, :, 3:4].broadcast_to([128, NSUB, 4]))
                nc.gpsimd.tensor_copy(
                    mk[:, :, 16:24].rearrange("p s (k g) -> p s g k", g=2),
                    gv[:, :, 4:6].broadcast_to([128, NSUB, 2, 4]))

                return xt, mk

            def mlp_phase(xt):
                """Block-diag 128x128 matmuls; returns p2 (PSUM)."""
                p2 = p2p.tile([128, NSUB, CH], F32)   # ((ht, s2), pi)
                p2f = p2.rearrange("p a b -> p (a b)")

                def emit_mm2(hidt, n0):
                    for q in range(2):
                        n = n0 + q
                        nc.tensor.matmul(
                            p2f,
                            lhsT=w2sb[:, n, :],
                            rhs=hidt[:, q].rearrange(
                                "p (ht pi s2) -> p ht s2 pi",
                                ht=8, s2=2),
                            start=(n == 0), stop=(n == NODES - 1),
                            skip_group_check=True)

                # software-pipelined: mm2 of half-wave k emits after the
                # mm1s of half-wave k+2, so drains never block the PE queue
                pend = []
                for k in range(2 * NW):
                    n0 = 2 * k
                    ph = php.tile([128, 2, 512], F32)
                    hidt = hp.tile([128, 2, 512], BF16)
                    for q in range(2):
                        nc.tensor.matmul(
                            ph[:, q, :],
                            lhsT=w1sb[:, n0 + q, :],
                            rhs=xt[:, :, n0 + q, :, :],
                            start=True, stop=True)
                    nc.scalar.activation(
                        hidt.rearrange("p a b -> p (a b)"),
                        ph.rearrange("p a b -> p (a b)"),
                        mybir.ActivationFunctionType.Relu,
                        bias=b1sb[:, n0:n0 + 1])
                    pend.append((hidt, n0))
                    if len(pend) > 2:
                        emit_mm2(*pend.pop(0))
                for args in pend:
                    emit_mm2(*args)
                return p2

            def output_phase(cc, p2, mk):
                """+b2, transpose back, mask, store chunk cc."""
                c0 = cc * C
                # p2 block f = (ht, s2) enumerates sub = hh*8 + 2t + s2
                # sequentially, so outT free is natural (sub, node) order.
                m2 = op.tile([128, NSUB, CH], F32)
                nc.scalar.activation(
                    m2.rearrange("p a b -> p (a b)"),
                    p2.rearrange("p a b -> p (a b)"),
                    mybir.ActivationFunctionType.Identity,
                    bias=b2sb[:, 0:1])
                outT = op.tile([128, NSUB, CH], F32)
                nc.vector.transpose(outT, m2)
                outF = op.tile([128, NSUB, NODES], F32)
                nc.gpsimd.tensor_tensor(
                    outF, outT[:, :, 0:NODES], mk, op=mybir.AluOpType.mult)
                for hh in range(2):
                    lo = c0 + hh * (C // 2)
                    nc.gpsimd.dma_start(
                        out=out[lo:lo + C // 2, :].rearrange(
                            "(p s) n -> p s n", p=128),
                        in_=outF[:, hh * HSUB:(hh + 1) * HSUB],
                    )

            for cc in range(NCHUNK):
                xt, mk = input_phase(cc)
                p2 = mlp_phase(xt)
                output_phase(cc, p2, mk)

    nc.compile()
    return nc


_PROGRAM = None


def _get_program():
    global _PROGRAM
    if _PROGRAM is None:
        _PROGRAM = _build_program()
    return _PROGRAM


def _prep_weights(W1, b1, W2, b2):
    flat = GROUPING.reshape(-1)
    g_of = np.zeros(NODES, np.int64)
    k_of = np.zeros(NODES, np.int64)
    for q, nid in enumerate(flat):
        g_of[nid] = q // 4
        k_of[nid] = q % 4
    W1n = np.ascontiguousarray(W1[g_of, k_of]).astype(np.float32)  # [24,32,32]
    W2n = np.ascontiguousarray(W2[g_of, k_of]).astype(np.float32)  # [24,32,1]
    b1n = np.ascontiguousarray(b1[g_of, k_of]).astype(np.float32)  # [24,32]
    b2n = np.ascontiguousarray(b2[g_of, k_of]).astype(np.float32)  # [24,1]

    w1dv = np.zeros((128, NODES, 128), np.float32)
    w2dv = np.zeros((128, NODES, 128), np.float32)
    b1cv = np.zeros((128, NODES), np.float32)
    b2cv = np.zeros((128, 1), np.float32)
    for n in range(NODES):
        for i in range(4):
            sl = slice(32 * i, 32 * i + 32)
            w1dv[sl, n, sl] = W1n[n]                       # [c, h] block
            w2dv[sl, n, 32 * i + n] = W2n[n][:, 0]         # column slot n
            b1cv[sl, n] = b1n[n]
            b2cv[32 * i + n, 0] = b2n[n, 0]
    return (w1dv.astype(ml_dtypes.bfloat16), w2dv.astype(ml_dtypes.bfloat16),
            b1cv, b2cv)


def _make_in_maps(inputs):
    w1dv, w2dv, b1cv, b2cv = _prep_weights(
        inputs["W1"], inputs["b1"], inputs["W2"], inputs["b2"])
    h2 = np.ascontiguousarray(inputs["h"], dtype=np.float32).reshape(
        S_TOT, NODES * CH)
    v2 = np.ascontiguousarray(inputs["valid"], dtype=np.int32).reshape(
        S_TOT, NODES)
    in_maps = []
    for c in range(N_CORES):
        sl = slice(c * S, (c + 1) * S)
        in_maps.append({
            "h": h2[sl],
            "valid": v2[sl],
            "w1d": w1dv,
            "w2d": w2dv,
            "b1c": b1cv,
            "b2c": b2cv,
        })
    return in_maps


def kernel(h, valid, W1, b1, W2, b2):
    nc = _get_program()
    in_maps = _make_in_maps(
        {"h": h, "valid": valid, "W1": W1, "b1": b1, "W2": W2, "b2": b2})
    res = run_bass_kernel_spmd(nc, in_maps, core_ids=list(range(N_CORES)))
    outs = [res.results[c]["out"] for c in range(N_CORES)]
    full = np.concatenate(outs, axis=0).astype(np.float32)
    return full.reshape(S_TOT, NODES, 1)
